# revision 32
# baseline (speedup 1.0000x reference)
"""AssociativeEmbeddingLoss on 8 TRN2 NeuronCores, v4.

Reference, per image b (C=1, G=128 boxes):
    tl[g] = pred[b, 0, ty[g], tx[g]],  br[g] = target[b, 0, by[g], bx[g]]
    me = (tl + br) / 2
    pull_b = sum((tl-br)^2) / (2N)
    push_b = sum_{i != j} relu(1 - |me_i - me_j|) / (N*(N-1))
    out = (0.25 * sum_b pull_b, 0.25 * sum_b push_b)

Data-parallel over batch, 8 images per core (2048 scattered scalars).
SWDGE descriptor generation runs at ~10.3ns/descriptor regardless of
how the gather is expressed (16x128 INDIRECT1D and 2x1024 DMAGatherAnt
both measure ~21us of Q7 time), so ~21us/core is the hard floor and
everything else must hide behind it. A 2-descriptor warm-up INDIRECT1D
runs on the idle Q7 during the framework preamble so the first real
gather skips first-use setup (~0.5us).

- match is DMA-loaded straight into the [128(g), 16(2b+tb)] layout via
  a 3-dim access pattern, so the 2048 flat offsets cost just 4 DVE ops
  (y*W, +x, +base(b,tb), int convert) - no PE transposes, and all 16
  gather columns are ready before the first gather issues.
- pred||target are concatenated host-side into one DRAM buffer (base
  for br columns includes +NPIX) so all 16 gathers read one tensor.
- the 16 [128,1] indirect gathers stream back-to-back on the GpSimd
  queue with no interleaved dependencies; per-image push compute
  (PE transpose -> K=1 bf16 ones x me_row matmul -> Scalar
  Abs(0.5x - me_i) with f32 bias -> DVE min(.,1)+accumulate, the DVE
  min lagging one image to avoid queue stalls) is pipelined two
  gathers (~2.7us) behind. The [128,128] pairwise tiles stay in PSUM
  end-to-end: Q7 descriptor generation shares SBUF read/write ports
  with DVE, and keeping the Abs writes + min reads off SBUF is what
  holds the gather cadence at ~1.4us/instr (it degrades ~25% and looks
  like bimodal run noise otherwise).
- pull reuses the gathered columns: dsub/sq as two bulk strided DVE
  ops into the same [128, 16] tile as the 8 min-accumulator columns,
  reduced by a single ones-matmul + two activation accumulators.
Each core returns [pull_partial, push_partial]; the host sums the 8
pairs (unshard).
"""

import numpy as np

import concourse.bacc as bacc
import concourse.mybir as mybir
import concourse.tile as tile
from concourse.bass import IndirectOffsetOnAxis
from concourse.bass_utils import run_bass_kernel_spmd

B, C, H, W = 64, 1, 512, 512
G = 128                 # boxes per image; N = G*C = 128
N = G * C
NCORES = 8
BP = B // NCORES        # images per core
NPIX = BP * H * W
M = 2 * BP              # gather columns: m = 2b + tb
PULL_W, PUSH_W = 0.25, 0.25

F32 = mybir.dt.float32
BF16 = mybir.dt.bfloat16
I32 = mybir.dt.int32
AF = mybir.ActivationFunctionType
ALU = mybir.AluOpType

C_PULL = PULL_W / (2.0 * N)
C_PUSH = PUSH_W / (N * (N - 1))

# cbig layout: [:, 0:128] identity; [:, 128] ones col;
# [0, 129:257] ones row; [:, 257:273] basepat
CB_ID = 0
CB_ONES = 128
CB_ONESROW = 129
CB_BASE = 257
CB_W = 273


def _build_nc():
    nc = bacc.Bacc(
        "TRN2",
        target_bir_lowering=False,
        debug=False,
        enable_asserts=False,
        num_devices=NCORES,
    )
    data = nc.dram_tensor("data", [2 * NPIX, 1], F32, kind="ExternalInput")
    match = nc.dram_tensor("match", [BP * G * 4, 1], F32, kind="ExternalInput")
    cbd = nc.dram_tensor("cbig", [G, CB_W], F32, kind="ExternalInput")
    out = nc.dram_tensor("out", [1, 2], F32, kind="ExternalOutput")

    with tile.TileContext(nc) as tc:
        _kernel_body(nc, tc, data, match, cbd, out)
    nc.compile()
    return nc


def _kernel_body(nc, tc, data, match, cbd, out):
    with (
        tc.tile_pool(name="sb", bufs=1) as sb,
        tc.tile_pool(name="ps", bufs=1, space="PSUM") as ps,
        tc.tile_pool(name="psr", bufs=2, space="PSUM") as psr,
    ):
        # ---- loads ----
        mg = sb.tile([G, 4 * BP], F32, tag="mg")
        msrc = match.ap()
        msrc.ap = mybir.VecI64Pair([[4, G], [4 * G, BP], [1, 4]])
        nc.sync.dma_start(out=mg[:], in_=msrc)
        cb = sb.tile([G, CB_W], F32, tag="cb")
        nc.scalar.dma_start(out=cb[:], in_=cbd.ap())
        ident = cb[:, CB_ID : CB_ID + G]
        ones = cb[:, CB_ONES : CB_ONES + 1]
        ones_row = cb[0:1, CB_ONESROW : CB_ONESROW + G]
        basepat = cb[:, CB_BASE : CB_BASE + M]

        # preload the activation table while DMAs are in flight
        scrd = sb.tile([1, 1], F32, tag="scrd")
        nc.scalar.activation(out=scrd[:], in_=cb[0:1, CB_ONES : CB_ONES + 1],
                             func=AF.Abs)

        # warm the INDIRECT1D path on the idle Q7 during the preamble
        zoff = sb.tile([2, 1], I32, tag="zoff")
        nc.gpsimd.memset(zoff[:], 0)
        wscrap = sb.tile([2, 1], F32, tag="wscrap")
        nc.gpsimd.indirect_dma_start(
            out=wscrap[:], out_offset=None, in_=data.ap(),
            in_offset=IndirectOffsetOnAxis(ap=zoff[:], axis=0),
        )



        # ---- flat offsets, native [128(g), 16(m)] layout ----
        # compute columns 0:2 first so gather 0 can issue ~0.6us earlier
        mgv = mg[:].rearrange("g (m yx) -> g m yx", m=M, yx=2)
        ft = sb.tile([G, M], F32, tag="ft")
        fti = sb.tile([G, M], I32, tag="fti")
        # column 0 (image 0 tl) has base offset 0: no basepat, so gather 0
        # depends only on the match load, not on the cbig constants DMA
        nc.vector.tensor_scalar(
            out=ft[:, 0:1], in0=mgv[:, 0, 0:1], scalar1=float(W), scalar2=None,
            op0=ALU.mult,
        )
        nc.vector.tensor_tensor(out=fti[:, 0:1], in0=ft[:, 0:1],
                                in1=mgv[:, 0, 1:2], op=ALU.add)
        cs = slice(1, M)
        nc.vector.tensor_scalar(
            out=ft[:, cs], in0=mgv[:, cs, 0], scalar1=float(W), scalar2=None,
            op0=ALU.mult,
        )
        nc.vector.tensor_tensor(out=ft[:, cs], in0=ft[:, cs],
                                in1=mgv[:, cs, 1], op=ALU.add)
        nc.vector.tensor_tensor(out=fti[:, cs], in0=ft[:, cs],
                                in1=basepat[:, cs], op=ALU.add)

        # bf16 stationary for the K=1 broadcast matmuls (4x PE rate);
        # placed after the offset chain so its cbig wait can't stall it
        ones16 = sb.tile([1, G], BF16, tag="ones16")
        nc.vector.tensor_copy(out=ones16[:], in_=cb[0:1, CB_ONESROW : CB_ONESROW + G])

        # ---- 16 gathers streaming on gpsimd; per-image push pipelined ----
        dcol = sb.tile([G, M], F32, tag="dcol")
        for m in range(M):
            nc.gpsimd.indirect_dma_start(
                out=dcol[:, m : m + 1], out_offset=None, in_=data.ap(),
                in_offset=IndirectOffsetOnAxis(ap=fti[:, m : m + 1], axis=0),
            )

        dv = dcol[:].rearrange("g (b t) -> g b t", b=BP, t=2)
        me = sb.tile([G, BP], F32, tag="me")
        negme = sb.tile([G, BP], F32, tag="negme")
        fin = sb.tile([G, 2 * BP], F32, tag="fin")   # cols 0:8 sq, 8:16 min

        def push_image(b):
            bs = slice(b, b + 1)
            nc.vector.tensor_tensor(out=me[:, bs], in0=dv[:, b, 0:1],
                                    in1=dv[:, b, 1:2], op=ALU.add)
            nc.vector.tensor_scalar(out=negme[:, bs], in0=me[:, bs],
                                    scalar1=-0.5, scalar2=None, op0=ALU.mult)
            rowp = psr.tile([1, G], F32, tag="rowp")
            nc.tensor.transpose(out=rowp[:], in_=me[:, bs], identity=ident)
            merow = sb.tile([1, G], BF16, tag=f"merow{b % 2}")
            nc.vector.tensor_copy(out=merow[:], in_=rowp[:])
            Rp = psr.tile([G, G], F32, tag="Rp")
            nc.tensor.matmul(out=Rp[:], lhsT=ones16[:], rhs=merow[:],
                             start=True, stop=True)
            # ad lives in PSUM: the Abs write and the min read stay off
            # the SBUF ports that Q7 descriptor generation contends on
            ad = psr.tile([G, G], F32, tag="ad")
            nc.scalar.activation(out=ad[:], in_=Rp[:], func=AF.Abs,
                                 bias=negme[:, bs], scale=0.5)
            return ad

        ads = [None, None]
        for b in range(BP):
            # lag the DVE min by one image so the vector queue never
            # stalls waiting on this image's ABS
            if b >= 1:
                pb = b - 1
                nc.vector.tensor_scalar(
                    out=ads[pb % 2][:], in0=ads[pb % 2][:], scalar1=1.0,
                    scalar2=0.0, op0=ALU.min, op1=ALU.add,
                    accum_out=fin[:, BP + pb : BP + pb + 1],
                )
            ads[b % 2] = push_image(b)

        # pull: bulk dsub/sq + its whole reduction run under the last ABS
        dsub = sb.tile([G, BP], F32, tag="dsub")
        nc.vector.tensor_tensor(out=dsub[:], in0=dv[:, :, 0], in1=dv[:, :, 1],
                                op=ALU.subtract)
        nc.vector.tensor_tensor(out=fin[:, 0:BP], in0=dsub[:], in1=dsub[:],
                                op=ALU.mult)
        pg = ps.tile([1, 2 * BP], F32, tag="pg")
        scr = sb.tile([1, 2 * BP], F32, tag="scr")
        res = sb.tile([1, 2], F32, tag="res")
        nc.tensor.matmul(out=pg[0:1, 0:BP], lhsT=ones, rhs=fin[:, 0:BP],
                         start=True, stop=True)
        nc.scalar.activation(out=scr[0:1, 0:BP], in_=pg[0:1, 0:BP], func=AF.Copy,
                             scale=C_PULL, accum_out=res[0:1, 0:1])
        nc.vector.tensor_scalar(
            out=ads[(BP - 1) % 2][:], in0=ads[(BP - 1) % 2][:], scalar1=1.0,
            scalar2=0.0, op0=ALU.min, op1=ALU.add,
            accum_out=fin[:, 2 * BP - 1 : 2 * BP],
        )

        # ---- push reduction: ones-matmul + accum activation ----
        nc.tensor.matmul(out=pg[0:1, BP : 2 * BP], lhsT=ones,
                         rhs=fin[:, BP : 2 * BP], start=True, stop=True)
        nc.scalar.activation(out=scr[0:1, BP : 2 * BP], in_=pg[0:1, BP : 2 * BP],
                             func=AF.Copy, scale=-C_PUSH,
                             bias=float(BP * N * (N - 1)) * C_PUSH / BP,
                             accum_out=res[0:1, 1:2])
        nc.sync.dma_start(out=out.ap(), in_=res[:])


_NC_CACHE = None


def _get_nc():
    global _NC_CACHE
    if _NC_CACHE is None:
        _NC_CACHE = _build_nc()
    return _NC_CACHE


def _consts():
    cb = np.zeros((G, CB_W), dtype=np.float32)
    cb[:, CB_ID : CB_ID + G] = np.eye(G, dtype=np.float32)
    cb[:, CB_ONES] = 1.0
    cb[0, CB_ONESROW : CB_ONESROW + G] = 1.0
    for q in range(M):
        cb[:, CB_BASE + q] = (q // 2) * H * W + (q % 2) * NPIX
    return cb


def make_in_maps(pred, target, match):
    pred = np.asarray(pred, dtype=np.float32).reshape(B, H * W)
    target = np.asarray(target, dtype=np.float32).reshape(B, H * W)
    match = np.asarray(match)
    cb = _consts()
    in_maps = []
    for k in range(NCORES):
        sl = slice(k * BP, (k + 1) * BP)
        data = np.concatenate(
            [pred[sl].reshape(-1), target[sl].reshape(-1)]
        ).reshape(2 * NPIX, 1)
        in_maps.append({
            "data": data,
            "match": np.ascontiguousarray(match[sl]).astype(np.float32).reshape(BP * G * 4, 1),
            "cbig": cb,
        })
    return in_maps


def kernel(pred, target, match, _trace=False):
    nc = _get_nc()
    in_maps = make_in_maps(pred, target, match)
    res = run_bass_kernel_spmd(nc, in_maps, core_ids=list(range(NCORES)), trace=_trace)
    total = np.zeros((1, 2), dtype=np.float64)
    for r in res.results:
        total += r["out"].astype(np.float64)
    out = (np.float32(total[0, 0]), np.float32(total[0, 1]))
    if _trace:
        return out, res
    return out


# revision 33
# speedup vs baseline: 1.1537x; 1.1537x over previous
"""AssociativeEmbeddingLoss on 8 TRN2 NeuronCores, v4.

Reference, per image b (C=1, G=128 boxes):
    tl[g] = pred[b, 0, ty[g], tx[g]],  br[g] = target[b, 0, by[g], bx[g]]
    me = (tl + br) / 2
    pull_b = sum((tl-br)^2) / (2N)
    push_b = sum_{i != j} relu(1 - |me_i - me_j|) / (N*(N-1))
    out = (0.25 * sum_b pull_b, 0.25 * sum_b push_b)

Data-parallel over batch, 8 images per core (2048 scattered scalars).
SWDGE descriptor generation runs at ~10.3ns/descriptor regardless of
how the gather is expressed (16x128 INDIRECT1D and 2x1024 DMAGatherAnt
both measure ~21us of Q7 time), so ~21us/core is the hard floor and
everything else must hide behind it. A 2-descriptor warm-up INDIRECT1D
runs on the idle Q7 during the framework preamble so the first real
gather skips first-use setup (~0.5us).

- match is DMA-loaded straight into the [128(g), 16(2b+tb)] layout via
  a 3-dim access pattern, so the 2048 flat offsets cost just 4 DVE ops
  (y*W, +x, +base(b,tb), int convert) - no PE transposes, and all 16
  gather columns are ready before the first gather issues.
- pred||target are concatenated host-side into one DRAM buffer (base
  for br columns includes +NPIX) so all 16 gathers read one tensor.
- the 16 [128,1] indirect gathers stream back-to-back on the GpSimd
  queue with no interleaved dependencies; per-image push compute
  (PE transpose -> K=1 bf16 ones x me_row matmul -> Scalar
  Abs(0.5x - me_i) with f32 bias -> DVE min(.,1)+accumulate, the DVE
  min lagging one image to avoid queue stalls) is pipelined two
  gathers (~2.7us) behind. The [128,128] pairwise tiles stay in PSUM
  end-to-end: Q7 descriptor generation shares SBUF read/write ports
  with DVE, and keeping the Abs writes + min reads off SBUF is what
  holds the gather cadence at ~1.4us/instr (it degrades ~25% and looks
  like bimodal run noise otherwise).
- pull reuses the gathered columns: dsub/sq as two bulk strided DVE
  ops into the same [128, 16] tile as the 8 min-accumulator columns,
  reduced by a single ones-matmul + two activation accumulators.
Each core returns [pull_partial, push_partial]; the host sums the 8
pairs (unshard).
"""

import numpy as np

import concourse.bacc as bacc
import concourse.mybir as mybir
import concourse.tile as tile
from concourse.bass import IndirectOffsetOnAxis
from concourse.bass_utils import run_bass_kernel_spmd

B, C, H, W = 64, 1, 512, 512
G = 128                 # boxes per image; N = G*C = 128
N = G * C
NCORES = 8
BP = B // NCORES        # images per core
NPIX = BP * H * W
M = 2 * BP              # gather columns: m = 2b + tb
PULL_W, PUSH_W = 0.25, 0.25

F32 = mybir.dt.float32
BF16 = mybir.dt.bfloat16
I32 = mybir.dt.int32
AF = mybir.ActivationFunctionType
ALU = mybir.AluOpType

C_PULL = PULL_W / (2.0 * N)
C_PUSH = PUSH_W / (N * (N - 1))

# cbig layout: [:, 0:128] identity; [:, 128] ones col;
# [0, 129:257] ones row; [:, 257:273] basepat
CB_ID = 0
CB_ONES = 128
CB_ONESROW = 129
CB_BASE = 257
CB_W = 273


def _build_nc():
    nc = bacc.Bacc(
        "TRN2",
        target_bir_lowering=False,
        debug=False,
        enable_asserts=False,
        num_devices=NCORES,
    )
    data = nc.dram_tensor("data", [2 * NPIX, 1], F32, kind="ExternalInput")
    match = nc.dram_tensor("match", [BP * G * 4, 1], F32, kind="ExternalInput")
    cbd = nc.dram_tensor("cbig", [G, CB_W], F32, kind="ExternalInput")
    out = nc.dram_tensor("out", [1, 2], F32, kind="ExternalOutput")

    with tile.TileContext(nc) as tc:
        _kernel_body(nc, tc, data, match, cbd, out)
    nc.compile()
    return nc


def _kernel_body(nc, tc, data, match, cbd, out):
    with (
        tc.tile_pool(name="sb", bufs=1) as sb,
        tc.tile_pool(name="ps", bufs=1, space="PSUM") as ps,
        tc.tile_pool(name="psr", bufs=2, space="PSUM") as psr,
    ):
        # ---- loads ----
        mg = sb.tile([G, 4 * BP], F32, tag="mg")
        msrc = match.ap()
        msrc.ap = mybir.VecI64Pair([[4, G], [4 * G, BP], [1, 4]])
        nc.sync.dma_start(out=mg[:], in_=msrc)
        cb = sb.tile([G, CB_W], F32, tag="cb")
        nc.scalar.dma_start(out=cb[:], in_=cbd.ap())
        ident = cb[:, CB_ID : CB_ID + G]
        ones = cb[:, CB_ONES : CB_ONES + 1]
        ones_row = cb[0:1, CB_ONESROW : CB_ONESROW + G]
        basepat = cb[:, CB_BASE : CB_BASE + M]

        # preload the activation table while DMAs are in flight
        scrd = sb.tile([1, 1], F32, tag="scrd")
        nc.scalar.activation(out=scrd[:], in_=cb[0:1, CB_ONES : CB_ONES + 1],
                             func=AF.Abs)

        # warm the INDIRECT1D path on the idle Q7 during the preamble
        zoff = sb.tile([2, 1], I32, tag="zoff")
        nc.gpsimd.memset(zoff[:], 0)
        wscrap = sb.tile([2, 1], F32, tag="wscrap")
        nc.gpsimd.indirect_dma_start(
            out=wscrap[:], out_offset=None, in_=data.ap(),
            in_offset=IndirectOffsetOnAxis(ap=zoff[:], axis=0),
        )



        # ---- flat offsets, native [128(g), 16(m)] layout ----
        # compute columns 0:2 first so gather 0 can issue ~0.6us earlier
        mgv = mg[:].rearrange("g (m yx) -> g m yx", m=M, yx=2)
        ft = sb.tile([G, M], F32, tag="ft")
        fti = sb.tile([G, M], I32, tag="fti")
        # column 0 (image 0 tl) has base offset 0: no basepat, so gather 0
        # depends only on the match load, not on the cbig constants DMA
        nc.vector.tensor_scalar(
            out=ft[:, 0:1], in0=mgv[:, 0, 0:1], scalar1=float(W), scalar2=None,
            op0=ALU.mult,
        )
        nc.vector.tensor_tensor(out=fti[:, 0:1], in0=ft[:, 0:1],
                                in1=mgv[:, 0, 1:2], op=ALU.add)
        cs = slice(1, M)
        nc.vector.tensor_scalar(
            out=ft[:, cs], in0=mgv[:, cs, 0], scalar1=float(W), scalar2=None,
            op0=ALU.mult,
        )
        nc.vector.tensor_tensor(out=ft[:, cs], in0=ft[:, cs],
                                in1=mgv[:, cs, 1], op=ALU.add)
        nc.vector.tensor_tensor(out=fti[:, cs], in0=ft[:, cs],
                                in1=basepat[:, cs], op=ALU.add)

        # bf16 stationary for the K=1 broadcast matmuls (4x PE rate);
        # placed after the offset chain so its cbig wait can't stall it
        ones16 = sb.tile([1, G], BF16, tag="ones16")
        nc.vector.tensor_copy(out=ones16[:], in_=cb[0:1, CB_ONESROW : CB_ONESROW + G])
        # bf16 identity: halves the 64KB moving-operand SBUF read each
        # per-image transpose streams during the gather phase
        ident16 = sb.tile([G, G], BF16, tag="ident16")
        nc.vector.tensor_copy(out=ident16[:], in_=ident)

        # ---- 16 gathers streaming on gpsimd; per-image push pipelined ----
        dcol = sb.tile([G, M], F32, tag="dcol")
        for m in range(M):
            nc.gpsimd.indirect_dma_start(
                out=dcol[:, m : m + 1], out_offset=None, in_=data.ap(),
                in_offset=IndirectOffsetOnAxis(ap=fti[:, m : m + 1], axis=0),
            )

        dv = dcol[:].rearrange("g (b t) -> g b t", b=BP, t=2)
        me = sb.tile([G, BP], BF16, tag="me")
        negme = sb.tile([G, BP], F32, tag="negme")
        fin = sb.tile([G, 2 * BP], F32, tag="fin")   # cols 0:8 sq, 8:16 min

        def push_image(b):
            bs = slice(b, b + 1)
            nc.vector.tensor_tensor(out=me[:, bs], in0=dv[:, b, 0:1],
                                    in1=dv[:, b, 1:2], op=ALU.add)
            nc.vector.tensor_scalar(out=negme[:, bs], in0=me[:, bs],
                                    scalar1=-0.5, scalar2=None, op0=ALU.mult)
            rowp = psr.tile([1, G], BF16, tag="rowp")
            nc.tensor.transpose(out=rowp[:], in_=me[:, bs], identity=ident16[:])
            merow = sb.tile([1, G], BF16, tag=f"merow{b % 2}")
            nc.vector.tensor_copy(out=merow[:], in_=rowp[:])
            Rp = psr.tile([G, G], F32, tag="Rp")
            nc.tensor.matmul(out=Rp[:], lhsT=ones16[:], rhs=merow[:],
                             start=True, stop=True)
            # ad lives in PSUM: the Abs write and the min read stay off
            # the SBUF ports that Q7 descriptor generation contends on
            ad = psr.tile([G, G], F32, tag="ad")
            nc.scalar.activation(out=ad[:], in_=Rp[:], func=AF.Abs,
                                 bias=negme[:, bs], scale=0.5)
            return ad

        ads = [None, None]
        for b in range(BP):
            # lag the DVE min by one image so the vector queue never
            # stalls waiting on this image's ABS
            if b >= 1:
                pb = b - 1
                nc.vector.tensor_scalar(
                    out=ads[pb % 2][:], in0=ads[pb % 2][:], scalar1=1.0,
                    scalar2=0.0, op0=ALU.min, op1=ALU.add,
                    accum_out=fin[:, BP + pb : BP + pb + 1],
                )
            ads[b % 2] = push_image(b)

        # pull: bulk dsub/sq + its whole reduction run under the last ABS
        dsub = sb.tile([G, BP], F32, tag="dsub")
        nc.vector.tensor_tensor(out=dsub[:], in0=dv[:, :, 0], in1=dv[:, :, 1],
                                op=ALU.subtract)
        nc.vector.tensor_tensor(out=fin[:, 0:BP], in0=dsub[:], in1=dsub[:],
                                op=ALU.mult)
        pg = ps.tile([1, 2 * BP], F32, tag="pg")
        scr = sb.tile([1, 2 * BP], F32, tag="scr")
        res = sb.tile([1, 2], F32, tag="res")
        nc.tensor.matmul(out=pg[0:1, 0:BP], lhsT=ones, rhs=fin[:, 0:BP],
                         start=True, stop=True)
        nc.scalar.activation(out=scr[0:1, 0:BP], in_=pg[0:1, 0:BP], func=AF.Copy,
                             scale=C_PULL, accum_out=res[0:1, 0:1])
        nc.vector.tensor_scalar(
            out=ads[(BP - 1) % 2][:], in0=ads[(BP - 1) % 2][:], scalar1=1.0,
            scalar2=0.0, op0=ALU.min, op1=ALU.add,
            accum_out=fin[:, 2 * BP - 1 : 2 * BP],
        )

        # ---- push reduction: ones-matmul + accum activation ----
        nc.tensor.matmul(out=pg[0:1, BP : 2 * BP], lhsT=ones,
                         rhs=fin[:, BP : 2 * BP], start=True, stop=True)
        nc.scalar.activation(out=scr[0:1, BP : 2 * BP], in_=pg[0:1, BP : 2 * BP],
                             func=AF.Copy, scale=-C_PUSH,
                             bias=float(BP * N * (N - 1)) * C_PUSH / BP,
                             accum_out=res[0:1, 1:2])
        nc.sync.dma_start(out=out.ap(), in_=res[:])


_NC_CACHE = None


def _get_nc():
    global _NC_CACHE
    if _NC_CACHE is None:
        _NC_CACHE = _build_nc()
    return _NC_CACHE


def _consts():
    cb = np.zeros((G, CB_W), dtype=np.float32)
    cb[:, CB_ID : CB_ID + G] = np.eye(G, dtype=np.float32)
    cb[:, CB_ONES] = 1.0
    cb[0, CB_ONESROW : CB_ONESROW + G] = 1.0
    for q in range(M):
        cb[:, CB_BASE + q] = (q // 2) * H * W + (q % 2) * NPIX
    return cb


def make_in_maps(pred, target, match):
    pred = np.asarray(pred, dtype=np.float32).reshape(B, H * W)
    target = np.asarray(target, dtype=np.float32).reshape(B, H * W)
    match = np.asarray(match)
    cb = _consts()
    in_maps = []
    for k in range(NCORES):
        sl = slice(k * BP, (k + 1) * BP)
        data = np.concatenate(
            [pred[sl].reshape(-1), target[sl].reshape(-1)]
        ).reshape(2 * NPIX, 1)
        in_maps.append({
            "data": data,
            "match": np.ascontiguousarray(match[sl]).astype(np.float32).reshape(BP * G * 4, 1),
            "cbig": cb,
        })
    return in_maps


def kernel(pred, target, match, _trace=False):
    nc = _get_nc()
    in_maps = make_in_maps(pred, target, match)
    res = run_bass_kernel_spmd(nc, in_maps, core_ids=list(range(NCORES)), trace=_trace)
    total = np.zeros((1, 2), dtype=np.float64)
    for r in res.results:
        total += r["out"].astype(np.float64)
    out = (np.float32(total[0, 0]), np.float32(total[0, 1]))
    if _trace:
        return out, res
    return out
